# revision 12
# baseline (speedup 1.0000x reference)
"""GroupWiseLinearProjector Trainium2 kernel.

Reference computation: x [B=16, C=768, H=64, W=64]; 16 spatial groups
(g = i*4+j owns pixels x[:, :, i::4, j::4]); per-group Linear
y_g = W_g @ x_pix + b_g with W_g [768, 768].

Strategy (8 NeuronCores, no cross-device comm):
  - Shard by GROUP: core c owns groups {2c, 2c+1}. Each core reads only
    its 2 weight matrices (resident in SBUF for the whole kernel) and
    exactly 1/8 of x / writes 1/8 of y -> minimal HBM traffic per core
    (~55 MB: 25.2 in + 25.2 out + 4.7 weights).
  - Host pre-gathers each group's strided pixels into SLAB-MAJOR layout
    [slab, partition, k, 1024] so a whole 3MB x-slab (and each y-slab
    store) moves with 24KB-per-partition contiguous DMA descriptors --
    the per-descriptor overhead is what caps effective HBM bandwidth.
  - Device: per group a dense GEMM [768x768] @ [768x4096], tiled
    128x128x512, fp32 data fed to the PE as float32r (FP22 multiply,
    FP32 accumulate) -> full PE rate at N=512. Loop order m->k->h so one
    stationary weight load feeds 2 matmuls.
  - Overlap: per-k chunk loads for the first two slabs (PE starts within
    ~3us and rides chunk arrivals), whole-slab single DMAs afterwards;
    x prefetched one slab ahead; loads on the Sync HWDGE queue, stores
    on the Scalar HWDGE queue so neither FIFO blocks the other.
"""

import sys

for _p in ("/opt/trn_rl_repo", "/root/.axon_site/_ro/trn_rl_repo"):
    if _p not in sys.path:
        sys.path.append(_p)

import numpy as np

B, C, H, W, G = 16, 768, 64, 64, 16
N_CORES = 8
GPC = G // N_CORES  # groups per core = 2
P = 128
KT = C // P   # 6 contraction tiles
MT = C // P   # 6 output-channel tiles
NPIX = B * (H // 4) * (W // 4)  # 4096 pixels per group
MMN = 512     # matmul moving free dim (one PSUM bank of fp32)
WIDTH = 1024  # n-slab width
SPG = NPIX // WIDTH  # 4 slabs per group
NSLAB = GPC * SPG    # 8 slabs per core
HB = WIDTH // MMN    # 2 psum banks per (m) block

MM_DTYPE_NAME = "float32r"

_CACHE = {}


def _build_nc():
    import concourse.mybir as mybir
    import concourse.tile as tile
    from concourse import bacc

    f32 = mybir.dt.float32
    mm_dt = getattr(mybir.dt, MM_DTYPE_NAME)

    nc = bacc.Bacc(None, target_bir_lowering=False)
    # x and W carry fp32 bits but are declared with the matmul dtype so the
    # BIR verifier sees a consistent FP32r producer/consumer chain (numpy
    # side both map to np.float32).
    xg = nc.dram_tensor("xg", [NSLAB, P, KT, WIDTH], mm_dt, kind="ExternalInput")
    wt = nc.dram_tensor("wt", [GPC, KT, P, C], mm_dt, kind="ExternalInput")
    bias = nc.dram_tensor("bias", [P, GPC * MT], f32, kind="ExternalInput")
    y = nc.dram_tensor("y", [NSLAB, P, MT, WIDTH], f32, kind="ExternalOutput")

    with tile.TileContext(nc) as tc:
        with (
            tc.tile_pool(name="wpool", bufs=1) as wpool,
            tc.tile_pool(name="bpool", bufs=1) as bpool,
            tc.tile_pool(name="xpool", bufs=3) as xpool,
            tc.tile_pool(name="opool", bufs=2) as opool,
            tc.tile_pool(name="pspool", bufs=8, space="PSUM") as pspool,
        ):
            w_tile = wpool.tile([P, GPC, KT, C], mm_dt)  # 36 KB/partition
            b_tile = bpool.tile([P, GPC, MT], f32)
            nc.sync.dma_start(b_tile[:], bias.rearrange("p (g mo) -> p g mo", g=GPC))

            def load_slab(i, chunked):
                t = xpool.tile([P, KT, WIDTH], mm_dt, tag="x")
                if chunked:
                    for k in range(KT):
                        nc.sync.dma_start(t[:, k, :], xg[i, :, k, :])
                else:
                    nc.sync.dma_start(t[:], xg[i])
                return t

            # Startup: group-0 weight chunks interleaved with slab 0's
            # chunks so the first matmul is gated on ~0.9MB, not 4.7MB.
            slab0 = xpool.tile([P, KT, WIDTH], mm_dt, tag="x")
            for k in range(KT):
                nc.sync.dma_start(w_tile[:, 0, k], wt[0, k])
                nc.sync.dma_start(slab0[:, k, :], xg[0, :, k, :])

            x_tiles = {0: slab0}
            for i in range(NSLAB):
                g = i // SPG
                if i + 1 < NSLAB:
                    x_tiles[i + 1] = load_slab(i + 1, chunked=(i + 1 < 2))
                # Stream group-1 weight chunks during the first iterations
                # (all landed long before slab 4 needs them).
                if GPC > 1 and i < 3:
                    nc.sync.dma_start(w_tile[:, 1, 2 * i], wt[1, 2 * i])
                    nc.sync.dma_start(w_tile[:, 1, 2 * i + 1], wt[1, 2 * i + 1])
                x_slab = x_tiles.pop(i)
                o_slab = opool.tile([P, MT, WIDTH], f32, tag="o")
                last = i == NSLAB - 1
                if i == 0:
                    # k-outer for the first slab: each k-step needs only the
                    # (w_k, x_k) chunk pair, so the PE streams behind the
                    # startup DMAs instead of idling until the slab lands.
                    for h in range(HB):
                        hs = slice(h * MMN, (h + 1) * MMN)
                        pk = [
                            pspool.tile([P, MMN], f32, tag="ps", name=f"ps0_{h}_{m}")
                            for m in range(MT)
                        ]
                        for k in range(KT):
                            for m in range(MT):
                                nc.tensor.matmul(
                                    pk[m][:],
                                    w_tile[:, g, k, m * P : (m + 1) * P],
                                    x_slab[:, k, hs],
                                    start=(k == 0),
                                    stop=(k == KT - 1),
                                )
                        for m in range(MT):
                            nc.vector.tensor_scalar_add(
                                o_slab[:, m, hs], pk[m][:], b_tile[:, g, m : m + 1]
                            )
                    nc.scalar.dma_start(y[i], o_slab[:])
                    continue
                for m in range(MT):
                    pss = [
                        pspool.tile([P, MMN], f32, tag="ps", name=f"ps_{i}_{m}_{h}")
                        for h in range(HB)
                    ]
                    for k in range(KT):
                        for h in range(HB):
                            # One stationary weight load feeds HB matmuls.
                            nc.tensor.matmul(
                                pss[h][:],
                                w_tile[:, g, k, m * P : (m + 1) * P],
                                x_slab[:, k, h * MMN : (h + 1) * MMN],
                                start=(k == 0),
                                stop=(k == KT - 1),
                            )
                    for h in range(HB):
                        # PSUM -> SBUF eviction fused with the bias add.
                        nc.vector.tensor_scalar_add(
                            o_slab[:, m, h * MMN : (h + 1) * MMN],
                            pss[h][:],
                            b_tile[:, g, m : m + 1],
                        )
                    if last:
                        # Per-m stores on the final slab shorten the drain.
                        nc.scalar.dma_start(y[i, :, m, :], o_slab[:, m, :])
                if not last:
                    # Whole-slab store: 24KB contiguous per partition.
                    nc.scalar.dma_start(y[i], o_slab[:])

    nc.compile()
    return nc


def _get_nc():
    if "nc" not in _CACHE:
        _CACHE["nc"] = _build_nc()
    return _CACHE["nc"]


def _shard_inputs(x, Wg, bg):
    """Host-side gather: slab-major dense blocks, one in_map per core."""
    x = np.ascontiguousarray(np.asarray(x, dtype=np.float32))
    Wg = np.asarray(Wg, dtype=np.float32)
    bg = np.asarray(bg, dtype=np.float32)

    # xt[i, j, c, b, hh, ww] = x[b, c, 4*hh+i, 4*ww+j]; group g = i*4+j.
    xt = np.ascontiguousarray(
        x.reshape(B, C, H // 4, 4, W // 4, 4).transpose(3, 5, 1, 0, 2, 4)
    ).reshape(G, C, NPIX)
    # [G, KT, P, SPG, WIDTH] -> slab-major [G, SPG, P, KT, WIDTH]
    xs = np.ascontiguousarray(
        xt.reshape(G, KT, P, SPG, WIDTH).transpose(0, 3, 2, 1, 4)
    )
    # wt[g, k, p, m] = Wg[g, m, k*128+p]
    wtT = np.ascontiguousarray(
        Wg.transpose(0, 2, 1).reshape(G, KT, P, C)
    )
    bias_arr = bg.reshape(G, MT, P).transpose(2, 0, 1)  # [P, G, MT]

    in_maps = []
    for c in range(N_CORES):
        gs = slice(GPC * c, GPC * (c + 1))
        in_maps.append(
            {
                "xg": xs[gs].reshape(NSLAB, P, KT, WIDTH),
                "wt": np.ascontiguousarray(wtT[gs]),
                "bias": np.ascontiguousarray(
                    bias_arr[:, gs, :].reshape(P, GPC * MT)
                ),
            }
        )
    return in_maps


def _unshard_output(results):
    """Host-side scatter of per-core [NSLAB, P, MT, WIDTH] to [B, C, H, W]."""
    yt = np.empty((4, 4, C, B, H // 4, W // 4), np.float32)
    for c in range(N_CORES):
        # [GPC, SPG, P, MT, WIDTH] -> [GPC, MT, P, SPG*WIDTH]
        yc = (
            np.asarray(results[c]["y"])
            .reshape(GPC, SPG, P, MT, WIDTH)
            .transpose(0, 3, 2, 1, 4)
            .reshape(GPC, C, B, H // 4, W // 4)
        )
        for gl in range(GPC):
            g = GPC * c + gl
            yt[g // 4, g % 4] = yc[gl]
    # yt[i, j, c, b, hh, ww] -> y[b, c, 4*hh+i, 4*ww+j]
    return np.ascontiguousarray(yt.transpose(3, 2, 4, 0, 5, 1)).reshape(B, C, H, W)


def run(x, Wg, bg, trace=False):
    from concourse.bass_utils import run_bass_kernel_spmd

    nc = _get_nc()
    in_maps = _shard_inputs(x, Wg, bg)
    res = run_bass_kernel_spmd(
        nc, in_maps, core_ids=list(range(N_CORES)), trace=trace
    )
    return _unshard_output(res.results), res


def kernel(x, Wg, bg):
    out, _ = run(x, Wg, bg, trace=False)
    return out


# revision 17
# speedup vs baseline: 1.0204x; 1.0204x over previous
"""GroupWiseLinearProjector Trainium2 kernel.

Reference computation: x [B=16, C=768, H=64, W=64]; 16 spatial groups
(g = i*4+j owns pixels x[:, :, i::4, j::4]); per-group Linear
y_g = W_g @ x_pix + b_g with W_g [768, 768].

Strategy (8 NeuronCores, no cross-device comm):
  - Shard by GROUP: core c owns groups {2c, 2c+1}. Each core reads only
    its 2 weight matrices (resident in SBUF for the whole kernel) and
    exactly 1/8 of x / writes 1/8 of y -> minimal HBM traffic per core
    (~55 MB: 25.2 in + 25.2 out + 4.7 weights).
  - Host pre-gathers each group's strided pixels into SLAB-MAJOR layout
    [slab, partition, k, 1024] so a whole 3MB x-slab (and each y-slab
    store) moves with 24KB-per-partition contiguous DMA descriptors --
    the per-descriptor overhead is what caps effective HBM bandwidth.
  - Device: per group a dense GEMM [768x768] @ [768x4096], tiled
    128x128x512, fp32 data fed to the PE as float32r (FP22 multiply,
    FP32 accumulate) -> full PE rate at N=512. Loop order m->k->h so one
    stationary weight load feeds 2 matmuls.
  - Overlap: per-k chunk loads for the first two slabs (PE starts within
    ~3us and rides chunk arrivals), whole-slab single DMAs afterwards;
    x prefetched one slab ahead; loads on the Sync HWDGE queue, stores
    on the Scalar HWDGE queue so neither FIFO blocks the other.
"""

import sys

for _p in ("/opt/trn_rl_repo", "/root/.axon_site/_ro/trn_rl_repo"):
    if _p not in sys.path:
        sys.path.append(_p)

import numpy as np

B, C, H, W, G = 16, 768, 64, 64, 16
N_CORES = 8
GPC = G // N_CORES  # groups per core = 2
P = 128
KT = C // P   # 6 contraction tiles
MT = C // P   # 6 output-channel tiles
NPIX = B * (H // 4) * (W // 4)  # 4096 pixels per group
MMN = 512     # matmul moving free dim (one PSUM bank of fp32)
WIDTH = 1024  # n-slab width
SPG = NPIX // WIDTH  # 4 slabs per group
NSLAB = GPC * SPG    # 8 slabs per core
HB = WIDTH // MMN    # 2 psum banks per (m) block

MM_DTYPE_NAME = "float32r"

_CACHE = {}


def _build_nc():
    import concourse.mybir as mybir
    import concourse.tile as tile
    from concourse import bacc

    f32 = mybir.dt.float32
    mm_dt = getattr(mybir.dt, MM_DTYPE_NAME)

    nc = bacc.Bacc(None, target_bir_lowering=False)
    # x and W carry fp32 bits but are declared with the matmul dtype so the
    # BIR verifier sees a consistent FP32r producer/consumer chain (numpy
    # side both map to np.float32).
    xg = nc.dram_tensor("xg", [NSLAB, P, KT, WIDTH], mm_dt, kind="ExternalInput")
    wt = nc.dram_tensor("wt", [GPC, KT, P, C], mm_dt, kind="ExternalInput")
    bias = nc.dram_tensor("bias", [P, GPC * MT], f32, kind="ExternalInput")
    y = nc.dram_tensor("y", [NSLAB, P, MT, WIDTH], f32, kind="ExternalOutput")

    with tile.TileContext(nc) as tc:
        with (
            tc.tile_pool(name="wpool", bufs=1) as wpool,
            tc.tile_pool(name="bpool", bufs=1) as bpool,
            tc.tile_pool(name="xpool", bufs=3) as xpool,
            tc.tile_pool(name="opool", bufs=3) as opool,
            tc.tile_pool(name="pspool", bufs=8, space="PSUM") as pspool,
        ):
            w_tile = wpool.tile([P, GPC, KT, C], mm_dt)  # 36 KB/partition
            b_tile = bpool.tile([P, GPC, MT], f32)
            nc.sync.dma_start(b_tile[:], bias.rearrange("p (g mo) -> p g mo", g=GPC))

            def load_slab(i, chunked):
                t = xpool.tile([P, KT, WIDTH], mm_dt, tag="x")
                if chunked:
                    for k in range(KT):
                        nc.sync.dma_start(t[:, k, :], xg[i, :, k, :])
                else:
                    nc.sync.dma_start(t[:], xg[i])
                return t

            # Startup: group-0 weight chunks interleaved with slab 0's
            # chunks so the first matmul is gated on ~0.9MB, not 4.7MB.
            slab0 = xpool.tile([P, KT, WIDTH], mm_dt, tag="x")
            for k in range(KT):
                nc.sync.dma_start(w_tile[:, 0, k], wt[0, k])
                nc.sync.dma_start(slab0[:, k, :], xg[0, :, k, :])

            x_tiles = {0: slab0}
            for i in range(NSLAB):
                g = i // SPG
                if i + 1 < NSLAB:
                    x_tiles[i + 1] = load_slab(
                        i + 1, chunked=(i + 1 < 2 or i + 1 == NSLAB - 1)
                    )
                # Stream group-1 weight chunks during the first iterations
                # (all landed long before slab 4 needs them).
                if GPC > 1 and i < 3:
                    nc.sync.dma_start(w_tile[:, 1, 2 * i], wt[1, 2 * i])
                    nc.sync.dma_start(w_tile[:, 1, 2 * i + 1], wt[1, 2 * i + 1])
                x_slab = x_tiles.pop(i)
                o_slab = opool.tile([P, MT, WIDTH], f32, tag="o")
                last = i == NSLAB - 1
                if i == 0 or last:
                    # k-outer for the first/last slab: each k-step needs only
                    # the k-chunks landed so far, so the PE streams behind the
                    # fill/drain DMAs instead of idling until the slab lands.
                    for h in range(HB):
                        hs = slice(h * MMN, (h + 1) * MMN)
                        pk = [
                            pspool.tile(
                                [P, MMN], f32, tag="ps", name=f"ps{i}_{h}_{m}"
                            )
                            for m in range(MT)
                        ]
                        for k in range(KT):
                            for m in range(MT):
                                nc.tensor.matmul(
                                    pk[m][:],
                                    w_tile[:, g, k, m * P : (m + 1) * P],
                                    x_slab[:, k, hs],
                                    start=(k == 0),
                                    stop=(k == KT - 1),
                                )
                        for m in range(MT):
                            nc.vector.tensor_scalar_add(
                                o_slab[:, m, hs], pk[m][:], b_tile[:, g, m : m + 1]
                            )
                            if last:
                                # Store each half row-block immediately so the
                                # drain is as short as possible.
                                nc.scalar.dma_start(
                                    y[i, :, m, hs], o_slab[:, m, hs]
                                )
                    if not last:
                        nc.scalar.dma_start(y[i], o_slab[:])
                    continue
                for m in range(MT):
                    pss = [
                        pspool.tile([P, MMN], f32, tag="ps", name=f"ps_{i}_{m}_{h}")
                        for h in range(HB)
                    ]
                    for k in range(KT):
                        for h in range(HB):
                            # One stationary weight load feeds HB matmuls.
                            nc.tensor.matmul(
                                pss[h][:],
                                w_tile[:, g, k, m * P : (m + 1) * P],
                                x_slab[:, k, h * MMN : (h + 1) * MMN],
                                start=(k == 0),
                                stop=(k == KT - 1),
                            )
                    for h in range(HB):
                        # PSUM -> SBUF eviction fused with the bias add.
                        nc.vector.tensor_scalar_add(
                            o_slab[:, m, h * MMN : (h + 1) * MMN],
                            pss[h][:],
                            b_tile[:, g, m : m + 1],
                        )
                    if last:
                        # Per-m stores on the final slab shorten the drain.
                        nc.scalar.dma_start(y[i, :, m, :], o_slab[:, m, :])
                if not last:
                    # Whole-slab store: 24KB contiguous per partition.
                    nc.scalar.dma_start(y[i], o_slab[:])

    nc.compile()
    return nc


def _get_nc():
    if "nc" not in _CACHE:
        _CACHE["nc"] = _build_nc()
    return _CACHE["nc"]


def _shard_inputs(x, Wg, bg):
    """Host-side gather: slab-major dense blocks, one in_map per core."""
    x = np.ascontiguousarray(np.asarray(x, dtype=np.float32))
    Wg = np.asarray(Wg, dtype=np.float32)
    bg = np.asarray(bg, dtype=np.float32)

    # xt[i, j, c, b, hh, ww] = x[b, c, 4*hh+i, 4*ww+j]; group g = i*4+j.
    xt = np.ascontiguousarray(
        x.reshape(B, C, H // 4, 4, W // 4, 4).transpose(3, 5, 1, 0, 2, 4)
    ).reshape(G, C, NPIX)
    # [G, KT, P, SPG, WIDTH] -> slab-major [G, SPG, P, KT, WIDTH]
    xs = np.ascontiguousarray(
        xt.reshape(G, KT, P, SPG, WIDTH).transpose(0, 3, 2, 1, 4)
    )
    # wt[g, k, p, m] = Wg[g, m, k*128+p]
    wtT = np.ascontiguousarray(
        Wg.transpose(0, 2, 1).reshape(G, KT, P, C)
    )
    bias_arr = bg.reshape(G, MT, P).transpose(2, 0, 1)  # [P, G, MT]

    in_maps = []
    for c in range(N_CORES):
        gs = slice(GPC * c, GPC * (c + 1))
        in_maps.append(
            {
                "xg": xs[gs].reshape(NSLAB, P, KT, WIDTH),
                "wt": np.ascontiguousarray(wtT[gs]),
                "bias": np.ascontiguousarray(
                    bias_arr[:, gs, :].reshape(P, GPC * MT)
                ),
            }
        )
    return in_maps


def _unshard_output(results):
    """Host-side scatter of per-core [NSLAB, P, MT, WIDTH] to [B, C, H, W]."""
    yt = np.empty((4, 4, C, B, H // 4, W // 4), np.float32)
    for c in range(N_CORES):
        # [GPC, SPG, P, MT, WIDTH] -> [GPC, MT, P, SPG*WIDTH]
        yc = (
            np.asarray(results[c]["y"])
            .reshape(GPC, SPG, P, MT, WIDTH)
            .transpose(0, 3, 2, 1, 4)
            .reshape(GPC, C, B, H // 4, W // 4)
        )
        for gl in range(GPC):
            g = GPC * c + gl
            yt[g // 4, g % 4] = yc[gl]
    # yt[i, j, c, b, hh, ww] -> y[b, c, 4*hh+i, 4*ww+j]
    return np.ascontiguousarray(yt.transpose(3, 2, 4, 0, 5, 1)).reshape(B, C, H, W)


def run(x, Wg, bg, trace=False):
    from concourse.bass_utils import run_bass_kernel_spmd

    nc = _get_nc()
    in_maps = _shard_inputs(x, Wg, bg)
    res = run_bass_kernel_spmd(
        nc, in_maps, core_ids=list(range(N_CORES)), trace=trace
    )
    return _unshard_output(res.results), res


def kernel(x, Wg, bg):
    out, _ = run(x, Wg, bg, trace=False)
    return out
